# revision 5
# baseline (speedup 1.0000x reference)
"""DETR scene-graph predicate head on 8 Trainium2 NeuronCores.

Math: logits[l,b,r,:] = concat(hs[l,b,q_sub], hs[l,b,q_obj]) @ W_pred.T + b_pred
where q_sub/q_obj come from (tgt_perm inverse, relationships, src_indices) —
pure integer index math, done on host.

Key structure: relations only ever reference the M=64 *matched* query slots
(pos = lookup[rel] in [0,64), q = src_indices[pos]), and the concat-linear
decomposes per relation into a sum of two halves:
  logits[r,p] = A[pos_sub(r),p] + B[pos_obj(r),p] + b
  A[k] = hs[src[k]] @ W1.T,  B[k] = hs[src[k]] @ W2.T
so the device computes the A|B tables for the 64 matched slots of every
(layer,image) block with dense batched matmuls — no gather, no one-hot on
device — and the host does the final O(L*B*R*P) index-select + add + bias,
the same class of index work it already does to build pos_sub/pos_obj.

Layout (batch axis sharded 8 ways; L*B/8 = 192 blocks/core, K=64 slots):
  - Host gathers hs[.,.,src[k],:] while packing (transposed, d on partitions)
    in groups of G=24 blocks: hst[g] = [128, (chunk, block, k)] bf16, 6KB DMA
    rows via the SWDGE (gpsimd) queue.
  - wpk [128, (chunk, 128)] is the stationary matmul operand: cols 0:51 W1.T,
    51:102 W2.T, 102:128 zero-pad so the output uses all 128 partitions
    (HWDGE splits DMAs by partition row across the 16 SDMA engines).
  - Per 8-block sub-batch: 2 accumulating matmuls (d-chunks) fill one full
    psum bank [128, 512] f32; one DVE/ACT cast (alternating) writes bf16 into
    the group output tile; one HWDGE store per group on the scalar queue.
  - A short dense-matmul preamble warms the PE clock while the first input
    group streams in.

hs and W are bf16 on-chip (psum accumulates f32); the AB tables round to bf16
on store. Host finishes in f32: rel err ~4e-3 vs the f32 reference.
"""

import sys

import numpy as np

L, B, Q1, D = 6, 256, 101, 256
M, R, P = 64, 64, 51
NCORES = 8
BLOC = B // NCORES          # images per core
NB = L * BLOC               # (layer, image) blocks per core
K = 64                      # matched slots per block (provably covers all rel)
G = 16                      # blocks per DMA group
NG = NB // G                # groups per core
SUB = 8                     # blocks per psum sub-batch
NSUB = G // SUB
CW = SUB * K                # psum cols per sub-batch (512 = full bank)
P2 = 2 * P                  # 102 live logit channels (sub | obj halves)

_CACHE = {}


def _build_program():
    import concourse.bacc as bacc
    import concourse.mybir as mybir
    import concourse.tile as tile
    from contextlib import ExitStack

    f32 = mybir.dt.float32
    bf16 = mybir.dt.bfloat16
    nc = bacc.Bacc("TRN2", target_bir_lowering=False, debug=False)

    GK = G * K
    hst = nc.dram_tensor("hst", [NG, 128, 2 * GK], bf16, kind="ExternalInput").ap()
    wpk = nc.dram_tensor("wpk", [128, 256], bf16, kind="ExternalInput").ap()
    outab = nc.dram_tensor("outab", [NG, 128, GK], bf16, kind="ExternalOutput").ap()

    with tile.TileContext(nc) as tc, ExitStack() as ctx:
        const = ctx.enter_context(tc.tile_pool(name="const", bufs=1))
        inp = ctx.enter_context(tc.tile_pool(name="inp", bufs=4))
        outp = ctx.enter_context(tc.tile_pool(name="outp", bufs=3))
        psA = ctx.enter_context(tc.tile_pool(name="psA", bufs=4, space="PSUM"))
        psW = ctx.enter_context(tc.tile_pool(name="psW", bufs=1, space="PSUM"))

        wpk_t = const.tile([128, 256], bf16)
        nc.sync.dma_start(out=wpk_t[:], in_=wpk[:])

        # HAM warm-up: dense N=512 matmuls nudge the PE clock up while the
        # first hst group streams in (no data dependency).
        wu = const.tile([128, 512], bf16)
        nc.vector.memset(wu[:], 0.0)
        wps = psW.tile([128, 512], f32, tag="wps")
        for _ in range(6):
            nc.tensor.matmul(out=wps[:], lhsT=wu[:, 0:128], rhs=wu[:],
                             start=True, stop=True)

        for g in range(NG):
            in_t = inp.tile([128, 2 * GK], bf16, tag="hst")
            nc.gpsimd.dma_start(out=in_t[:], in_=hst[g])
            o_t = outp.tile([128, GK], bf16, tag="o")

            for sb in range(NSUB):
                ps = psA.tile([128, CW], f32, tag="ps")
                lo = sb * CW
                nc.tensor.matmul(out=ps[:],
                                 lhsT=wpk_t[:, 0:128],
                                 rhs=in_t[:, lo:lo + CW],
                                 start=True, stop=False)
                nc.tensor.matmul(out=ps[:],
                                 lhsT=wpk_t[:, 128:256],
                                 rhs=in_t[:, GK + lo:GK + lo + CW],
                                 start=False, stop=True)
                nc.vector.tensor_copy(out=o_t[:, lo:lo + CW], in_=ps[:])

            nc.scalar.dma_start(out=outab[g], in_=o_t[:])

    nc.compile()
    return nc


def _host_indices(src_indices, tgt_perm, relationships):
    """pos_sub, pos_obj: [L, B, R] int64 — matched-slot index per relation."""
    tgt = np.asarray(tgt_perm, dtype=np.int64)
    rel = np.asarray(relationships, dtype=np.int64)

    # lookup[l, b, tgt[l, b, k]] = k
    lookup = np.empty((L, B, M), dtype=np.int64)
    li = np.arange(L)[:, None, None]
    bi = np.arange(B)[None, :, None]
    lookup[li, bi, tgt] = np.broadcast_to(np.arange(M), (L, B, M))

    sub_t = np.broadcast_to(rel[None, :, :, 0], (L, B, R))
    obj_t = np.broadcast_to(rel[None, :, :, 1], (L, B, R))
    pos_sub = np.take_along_axis(lookup, sub_t, axis=2)
    pos_obj = np.take_along_axis(lookup, obj_t, axis=2)
    return pos_sub, pos_obj


def _host_prepare(hs, src_indices, tgt_perm, relationships, W_pred, b_pred):
    """Build per-core input maps."""
    import ml_dtypes
    bf16 = ml_dtypes.bfloat16

    hs = np.asarray(hs, dtype=np.float32)
    W = np.asarray(W_pred, dtype=np.float32)
    src = np.asarray(src_indices, dtype=np.int64)

    # Wpad [256, 128]: cols 0:51 = W1.T, 51:102 = W2.T, rest zero
    wpad = np.zeros((D, 128), dtype=np.float32)
    wpad[:, :P] = W[:, :D].T
    wpad[:, P:P2] = W[:, D:].T
    wpk = np.ascontiguousarray(
        wpad.reshape(2, 128, 128).transpose(1, 0, 2).reshape(128, 256)
    ).astype(bf16)

    hs_bf = hs.astype(bf16)
    in_maps = []
    for c in range(NCORES):
        sl = slice(c * BLOC, (c + 1) * BLOC)
        # matched-slot table: hs_m[l, i, k, d] = hs[l, b, src[l, b, k], d]
        hs_m = np.take_along_axis(hs_bf[:, sl], src[:, sl, :, None], axis=2)
        # [NB, K, 2, 128] -> [NG, 128, 2, G, K]
        hst = np.ascontiguousarray(
            hs_m.reshape(NG, G, K, 2, 128).transpose(0, 4, 3, 1, 2)
            .reshape(NG, 128, 2 * G * K))
        in_maps.append({"hst": hst, "wpk": wpk})
    return in_maps


def kernel(hs, src_indices, tgt_perm, relationships, W_pred, b_pred):
    if "concourse" not in sys.modules:
        try:
            import concourse  # noqa: F401
        except ImportError:
            sys.path.insert(0, "/opt/trn_rl_repo")
    from concourse import bass_utils

    in_maps = _host_prepare(hs, src_indices, tgt_perm, relationships,
                            W_pred, b_pred)
    if "nc" not in _CACHE:
        _CACHE["nc"] = _build_program()
    nc = _CACHE["nc"]

    res = bass_utils.run_bass_kernel_spmd(nc, in_maps, list(range(NCORES)))

    pos_sub, pos_obj = _host_indices(src_indices, tgt_perm, relationships)
    b = np.asarray(b_pred, dtype=np.float32)

    outs = []
    for c in range(NCORES):
        ab = res.results[c]["outab"]                  # [NG, 128, G*K] bf16
        ab = ab.astype(np.float32).reshape(NG, 128, G, K)
        ab = ab.transpose(0, 2, 1, 3).reshape(L, BLOC, 128, K)
        sl = slice(c * BLOC, (c + 1) * BLOC)
        ks = pos_sub[:, sl, None, :]                  # [L, BLOC, 1, R]
        ko = pos_obj[:, sl, None, :]
        a_half = np.take_along_axis(ab[:, :, :P, :],
                                    np.broadcast_to(ks, (L, BLOC, P, R)),
                                    axis=3)           # [L, BLOC, P, R]
        b_half = np.take_along_axis(ab[:, :, P:P2, :],
                                    np.broadcast_to(ko, (L, BLOC, P, R)),
                                    axis=3)
        logits = (a_half + b_half).transpose(0, 1, 3, 2) + b
        outs.append(np.ascontiguousarray(logits))
    return np.concatenate(outs, axis=1)


# revision 6
# speedup vs baseline: 1.1431x; 1.1431x over previous
"""DETR scene-graph predicate head on 8 Trainium2 NeuronCores.

Math: logits[l,b,r,:] = concat(hs[l,b,q_sub], hs[l,b,q_obj]) @ W_pred.T + b_pred
where q_sub/q_obj come from (tgt_perm inverse, relationships, src_indices) —
pure integer index math, done on host.

Key structure: relations only reference matched query slots (pos in [0,64),
q = src_indices[pos]), so at most 64 — measured ~43 — distinct queries per
(layer,image) block ever matter. The concat-linear decomposes per relation:
  logits[r,p] = A[q_sub(r),p] + B[q_obj(r),p] + b,  A = hs@W1.T, B = hs@W2.T
so the device computes the A|B tables over each block's distinct used
queries (padded to KR slots) with dense batched matmuls — no gather and no
one-hot on device — and the host does the final O(L*B*R*P) index-select +
add + bias, the same class of index work it already does for pos_sub/pos_obj.

Layout (batch axis sharded 8 ways; L*B/8 = 192 blocks/core):
  - Host packs the distinct-query hs columns (transposed, d on partitions) in
    groups of G=24 blocks: hst[g] = [128, (chunk, block, slot)] bf16, ~5KB DMA
    rows via the SWDGE (gpsimd) queue.
  - wpk [128, (chunk, 128)] is the stationary matmul operand: cols 0:51 W1.T,
    51:102 W2.T, 102:128 zero-pad so the output tile has 128 live partition
    rows (HWDGE splits DMAs by partition row across the 16 SDMA engines).
  - Per 8-block sub-batch: 2 accumulating matmuls (d-chunks) fill one psum
    bank [128, 8*KR] f32; one DVE/ACT cast (alternating) writes bf16 into the
    group output tile; one HWDGE store per group on the scalar queue.
  - A short dense-matmul preamble warms the PE clock while the first input
    group streams in.

hs and W are bf16 on-chip (psum accumulates f32); the AB tables round to bf16
on store. Host finishes in f32: rel err ~3e-3 vs the f32 reference.
"""

import sys

import numpy as np

L, B, Q1, D = 6, 256, 101, 256
M, R, P = 64, 64, 51
NCORES = 8
BLOC = B // NCORES          # images per core
NB = L * BLOC               # (layer, image) blocks per core
G = 24                      # blocks per DMA group
NG = NB // G                # groups per core
SUB = 8                     # blocks per psum sub-batch
NSUB = G // SUB
P2 = 2 * P                  # 102 live logit channels (sub | obj halves)

_CACHE = {}


def _build_program(KR):
    import concourse.bacc as bacc
    import concourse.mybir as mybir
    import concourse.tile as tile
    from contextlib import ExitStack

    f32 = mybir.dt.float32
    bf16 = mybir.dt.bfloat16
    nc = bacc.Bacc("TRN2", target_bir_lowering=False, debug=False)

    GK = G * KR
    CW = SUB * KR           # psum cols per sub-batch (<= 512)
    hst = nc.dram_tensor("hst", [NG, 128, 2 * GK], bf16, kind="ExternalInput").ap()
    wpk = nc.dram_tensor("wpk", [128, 256], bf16, kind="ExternalInput").ap()
    outab = nc.dram_tensor("outab", [NG, 128, GK], bf16, kind="ExternalOutput").ap()

    with tile.TileContext(nc) as tc, ExitStack() as ctx:
        const = ctx.enter_context(tc.tile_pool(name="const", bufs=1))
        inp = ctx.enter_context(tc.tile_pool(name="inp", bufs=3))
        outp = ctx.enter_context(tc.tile_pool(name="outp", bufs=3))
        psA = ctx.enter_context(tc.tile_pool(name="psA", bufs=4, space="PSUM"))
        psW = ctx.enter_context(tc.tile_pool(name="psW", bufs=1, space="PSUM"))

        wpk_t = const.tile([128, 256], bf16)
        nc.sync.dma_start(out=wpk_t[:], in_=wpk[:])

        # HAM warm-up: dense N=512 matmuls nudge the PE clock up while the
        # first hst group streams in (no data dependency).
        wu = const.tile([128, 512], bf16)
        nc.vector.memset(wu[:], 0.0)
        wps = psW.tile([128, 512], f32, tag="wps")
        for _ in range(10):
            nc.tensor.matmul(out=wps[:], lhsT=wu[:, 0:128], rhs=wu[:],
                             start=True, stop=True)

        cast_flip = 0
        for g in range(NG):
            in_t = inp.tile([128, 2 * GK], bf16, tag="hst")
            nc.gpsimd.dma_start(out=in_t[:], in_=hst[g])
            o_t = outp.tile([128, GK], bf16, tag="o")

            for sb in range(NSUB):
                ps = psA.tile([128, CW], f32, tag="ps")
                lo = sb * CW
                nc.tensor.matmul(out=ps[:],
                                 lhsT=wpk_t[:, 0:128],
                                 rhs=in_t[:, lo:lo + CW],
                                 start=True, stop=False)
                nc.tensor.matmul(out=ps[:],
                                 lhsT=wpk_t[:, 128:256],
                                 rhs=in_t[:, GK + lo:GK + lo + CW],
                                 start=False, stop=True)
                if cast_flip == 0:
                    nc.vector.tensor_copy(out=o_t[:, lo:lo + CW], in_=ps[:])
                else:
                    nc.scalar.copy(out=o_t[:, lo:lo + CW], in_=ps[:])
                cast_flip ^= 1

            nc.scalar.dma_start(out=outab[g], in_=o_t[:])

    nc.compile()
    return nc


def _host_indices(src_indices, tgt_perm, relationships):
    """q_sub, q_obj: [L, B, R] int64 — query slot per relation."""
    src = np.asarray(src_indices, dtype=np.int64)
    tgt = np.asarray(tgt_perm, dtype=np.int64)
    rel = np.asarray(relationships, dtype=np.int64)

    # lookup[l, b, tgt[l, b, k]] = k
    lookup = np.empty((L, B, M), dtype=np.int64)
    li = np.arange(L)[:, None, None]
    bi = np.arange(B)[None, :, None]
    lookup[li, bi, tgt] = np.broadcast_to(np.arange(M), (L, B, M))

    sub_t = np.broadcast_to(rel[None, :, :, 0], (L, B, R))
    obj_t = np.broadcast_to(rel[None, :, :, 1], (L, B, R))
    pos_sub = np.take_along_axis(lookup, sub_t, axis=2)
    pos_obj = np.take_along_axis(lookup, obj_t, axis=2)
    q_sub = np.take_along_axis(src, pos_sub, axis=2)
    q_obj = np.take_along_axis(src, pos_obj, axis=2)
    return q_sub, q_obj


def _compaction(q_sub, q_obj):
    """Per-block distinct-query compaction.

    Returns (KR, uq_pad [L,B,KR] — the distinct queries per block (padded
    with arbitrary valid q), slot [L,B,Q1] — query -> compact slot).
    """
    used = np.zeros((L * B, Q1), dtype=bool)
    rows = np.arange(L * B)[:, None]
    qcat = np.concatenate([q_sub, q_obj], axis=-1).reshape(L * B, 2 * R)
    used[rows, qcat] = True
    nuniq = used.sum(axis=1)
    KR = min(64, max(8, int(-(-nuniq.max() // 8) * 8)))
    # stable argsort of ~used: first nuniq entries = used queries, ascending
    order = np.argsort(~used, axis=1, kind="stable")
    uq_pad = order[:, :KR].reshape(L, B, KR)
    slot = np.cumsum(used, axis=1) - 1
    slot = slot.reshape(L, B, Q1)
    return KR, uq_pad, slot


def _host_prepare(hs, src_indices, tgt_perm, relationships, W_pred, b_pred):
    """Build (KR, per-core input maps, gather indices)."""
    import ml_dtypes
    bf16 = ml_dtypes.bfloat16

    hs = np.asarray(hs, dtype=np.float32)
    W = np.asarray(W_pred, dtype=np.float32)

    q_sub, q_obj = _host_indices(src_indices, tgt_perm, relationships)
    KR, uq_pad, slot = _compaction(q_sub, q_obj)
    j_sub = np.take_along_axis(slot, q_sub, axis=2)   # [L, B, R] compact idx
    j_obj = np.take_along_axis(slot, q_obj, axis=2)

    # Wpad [256, 128]: cols 0:51 = W1.T, 51:102 = W2.T, rest zero
    wpad = np.zeros((D, 128), dtype=np.float32)
    wpad[:, :P] = W[:, :D].T
    wpad[:, P:P2] = W[:, D:].T
    wpk = np.ascontiguousarray(
        wpad.reshape(2, 128, 128).transpose(1, 0, 2).reshape(128, 256)
    ).astype(bf16)

    hs_bf = hs.astype(bf16)
    in_maps = []
    for c in range(NCORES):
        sl = slice(c * BLOC, (c + 1) * BLOC)
        # compacted table: hs_m[l, i, j, d] = hs[l, b, uq_pad[l, b, j], d]
        hs_m = np.take_along_axis(hs_bf[:, sl], uq_pad[:, sl, :, None], axis=2)
        # [NB, KR, 2, 128] -> [NG, 128, 2, G, KR]
        hst = np.ascontiguousarray(
            hs_m.reshape(NG, G, KR, 2, 128).transpose(0, 4, 3, 1, 2)
            .reshape(NG, 128, 2 * G * KR))
        in_maps.append({"hst": hst, "wpk": wpk})
    return KR, in_maps, j_sub, j_obj


def kernel(hs, src_indices, tgt_perm, relationships, W_pred, b_pred):
    if "concourse" not in sys.modules:
        try:
            import concourse  # noqa: F401
        except ImportError:
            sys.path.insert(0, "/opt/trn_rl_repo")
    from concourse import bass_utils

    KR, in_maps, j_sub, j_obj = _host_prepare(
        hs, src_indices, tgt_perm, relationships, W_pred, b_pred)
    if _CACHE.get("KR") != KR:
        _CACHE["nc"] = _build_program(KR)
        _CACHE["KR"] = KR
    nc = _CACHE["nc"]

    res = bass_utils.run_bass_kernel_spmd(nc, in_maps, list(range(NCORES)))

    b = np.asarray(b_pred, dtype=np.float32)
    outs = []
    for c in range(NCORES):
        ab = res.results[c]["outab"]                  # [NG, 128, G*KR] bf16
        ab = ab.astype(np.float32).reshape(NG, 128, G, KR)
        ab = ab.transpose(0, 2, 1, 3).reshape(L, BLOC, 128, KR)
        sl = slice(c * BLOC, (c + 1) * BLOC)
        ks = j_sub[:, sl, None, :]                    # [L, BLOC, 1, R]
        ko = j_obj[:, sl, None, :]
        a_half = np.take_along_axis(ab[:, :, :P, :],
                                    np.broadcast_to(ks, (L, BLOC, P, R)),
                                    axis=3)           # [L, BLOC, P, R]
        b_half = np.take_along_axis(ab[:, :, P:P2, :],
                                    np.broadcast_to(ko, (L, BLOC, P, R)),
                                    axis=3)
        logits = (a_half + b_half).transpose(0, 1, 3, 2) + b
        outs.append(np.ascontiguousarray(logits))
    return np.concatenate(outs, axis=1)


# revision 7
# speedup vs baseline: 1.1984x; 1.0483x over previous
"""DETR scene-graph predicate head on 8 Trainium2 NeuronCores.

Math: logits[l,b,r,:] = concat(hs[l,b,q_sub], hs[l,b,q_obj]) @ W_pred.T + b_pred
where q_sub/q_obj come from (tgt_perm inverse, relationships, src_indices) —
pure integer index math, done on host.

Key structure: relations only reference matched query slots (pos in [0,64),
q = src_indices[pos]), so at most 64 — measured ~43 — distinct queries per
(layer,image) block ever matter. The concat-linear decomposes per relation:
  logits[r,p] = A[q_sub(r),p] + B[q_obj(r),p] + b,  A = hs@W1.T, B = hs@W2.T
so the device computes the A|B tables over each block's distinct used
queries (padded to KR slots) with dense batched matmuls — no gather and no
one-hot on device — and the host does the final O(L*B*R*P) index-select +
add + bias, the same class of index work it already does for pos_sub/pos_obj.

Layout (batch axis sharded 8 ways; L*B/8 = 192 blocks/core):
  - Host packs the distinct-query hs columns (transposed, d on partitions) in
    groups of G=24 blocks: hst[g] = [128, (chunk, block, slot)] bf16, ~5KB DMA
    rows via the SWDGE (gpsimd) queue.
  - wpk [128, (chunk, 128)] is the stationary matmul operand: cols 0:51 W1.T,
    51:102 W2.T, 102:128 zero-pad so the output tile has 128 live partition
    rows (HWDGE splits DMAs by partition row across the 16 SDMA engines).
  - Per 8-block sub-batch: 2 accumulating matmuls (d-chunks) fill one psum
    bank [128, 8*KR] f32; one DVE/ACT cast (alternating) writes bf16 into the
    group output tile; one HWDGE store per group on the scalar queue.
  - A short dense-matmul preamble warms the PE clock while the first input
    group streams in.

hs and W are bf16 on-chip (psum accumulates f32); the AB tables round to bf16
on store. Host finishes in f32: rel err ~3e-3 vs the f32 reference.
"""

import sys

import numpy as np

L, B, Q1, D = 6, 256, 101, 256
M, R, P = 64, 64, 51
NCORES = 8
BLOC = B // NCORES          # images per core
NB = L * BLOC               # (layer, image) blocks per core
G = 48                      # blocks per DMA group
NG = NB // G                # groups per core
SUB = 8                     # blocks per psum sub-batch
NSUB = G // SUB
P2 = 2 * P                  # 102 live logit channels (sub | obj halves)

_CACHE = {}


def _build_program(KR):
    import concourse.bacc as bacc
    import concourse.mybir as mybir
    import concourse.tile as tile
    from contextlib import ExitStack

    f32 = mybir.dt.float32
    bf16 = mybir.dt.bfloat16
    nc = bacc.Bacc("TRN2", target_bir_lowering=False, debug=False)

    GK = G * KR
    CW = SUB * KR           # psum cols per sub-batch (<= 512)
    hst = nc.dram_tensor("hst", [NG, 128, 2 * GK], bf16, kind="ExternalInput").ap()
    wpk = nc.dram_tensor("wpk", [128, 256], bf16, kind="ExternalInput").ap()
    outab = nc.dram_tensor("outab", [NG, 128, GK], bf16, kind="ExternalOutput").ap()

    with tile.TileContext(nc) as tc, ExitStack() as ctx:
        const = ctx.enter_context(tc.tile_pool(name="const", bufs=1))
        inp = ctx.enter_context(tc.tile_pool(name="inp", bufs=3))
        outp = ctx.enter_context(tc.tile_pool(name="outp", bufs=3))
        psA = ctx.enter_context(tc.tile_pool(name="psA", bufs=4, space="PSUM"))
        psW = ctx.enter_context(tc.tile_pool(name="psW", bufs=1, space="PSUM"))

        wpk_t = const.tile([128, 256], bf16)
        nc.sync.dma_start(out=wpk_t[:], in_=wpk[:])

        # HAM warm-up: dense N=512 matmuls nudge the PE clock up while the
        # first hst group streams in (no data dependency).
        wu = const.tile([128, 512], bf16)
        nc.vector.memset(wu[:], 0.0)
        wps = psW.tile([128, 512], f32, tag="wps")
        for _ in range(10):
            nc.tensor.matmul(out=wps[:], lhsT=wu[:, 0:128], rhs=wu[:],
                             start=True, stop=True)

        cast_flip = 0
        for g in range(NG):
            in_t = inp.tile([128, 2 * GK], bf16, tag="hst")
            nc.gpsimd.dma_start(out=in_t[:], in_=hst[g])
            o_t = outp.tile([128, GK], bf16, tag="o")

            for sb in range(NSUB):
                ps = psA.tile([128, CW], f32, tag="ps")
                lo = sb * CW
                nc.tensor.matmul(out=ps[:],
                                 lhsT=wpk_t[:, 0:128],
                                 rhs=in_t[:, lo:lo + CW],
                                 start=True, stop=False)
                nc.tensor.matmul(out=ps[:],
                                 lhsT=wpk_t[:, 128:256],
                                 rhs=in_t[:, GK + lo:GK + lo + CW],
                                 start=False, stop=True)
                if cast_flip == 0:
                    nc.vector.tensor_copy(out=o_t[:, lo:lo + CW], in_=ps[:])
                else:
                    nc.scalar.copy(out=o_t[:, lo:lo + CW], in_=ps[:])
                cast_flip ^= 1

            nc.scalar.dma_start(out=outab[g], in_=o_t[:])

    nc.compile()
    return nc


def _host_indices(src_indices, tgt_perm, relationships):
    """q_sub, q_obj: [L, B, R] int64 — query slot per relation."""
    src = np.asarray(src_indices, dtype=np.int64)
    tgt = np.asarray(tgt_perm, dtype=np.int64)
    rel = np.asarray(relationships, dtype=np.int64)

    # lookup[l, b, tgt[l, b, k]] = k
    lookup = np.empty((L, B, M), dtype=np.int64)
    li = np.arange(L)[:, None, None]
    bi = np.arange(B)[None, :, None]
    lookup[li, bi, tgt] = np.broadcast_to(np.arange(M), (L, B, M))

    sub_t = np.broadcast_to(rel[None, :, :, 0], (L, B, R))
    obj_t = np.broadcast_to(rel[None, :, :, 1], (L, B, R))
    pos_sub = np.take_along_axis(lookup, sub_t, axis=2)
    pos_obj = np.take_along_axis(lookup, obj_t, axis=2)
    q_sub = np.take_along_axis(src, pos_sub, axis=2)
    q_obj = np.take_along_axis(src, pos_obj, axis=2)
    return q_sub, q_obj


def _compaction(q_sub, q_obj):
    """Per-block distinct-query compaction.

    Returns (KR, uq_pad [L,B,KR] — the distinct queries per block (padded
    with arbitrary valid q), slot [L,B,Q1] — query -> compact slot).
    """
    used = np.zeros((L * B, Q1), dtype=bool)
    rows = np.arange(L * B)[:, None]
    qcat = np.concatenate([q_sub, q_obj], axis=-1).reshape(L * B, 2 * R)
    used[rows, qcat] = True
    nuniq = used.sum(axis=1)
    KR = min(64, max(8, int(-(-nuniq.max() // 8) * 8)))
    # stable argsort of ~used: first nuniq entries = used queries, ascending
    order = np.argsort(~used, axis=1, kind="stable")
    uq_pad = order[:, :KR].reshape(L, B, KR)
    slot = np.cumsum(used, axis=1) - 1
    slot = slot.reshape(L, B, Q1)
    return KR, uq_pad, slot


def _host_prepare(hs, src_indices, tgt_perm, relationships, W_pred, b_pred):
    """Build (KR, per-core input maps, gather indices)."""
    import ml_dtypes
    bf16 = ml_dtypes.bfloat16

    hs = np.asarray(hs, dtype=np.float32)
    W = np.asarray(W_pred, dtype=np.float32)

    q_sub, q_obj = _host_indices(src_indices, tgt_perm, relationships)
    KR, uq_pad, slot = _compaction(q_sub, q_obj)
    j_sub = np.take_along_axis(slot, q_sub, axis=2)   # [L, B, R] compact idx
    j_obj = np.take_along_axis(slot, q_obj, axis=2)

    # Wpad [256, 128]: cols 0:51 = W1.T, 51:102 = W2.T, rest zero
    wpad = np.zeros((D, 128), dtype=np.float32)
    wpad[:, :P] = W[:, :D].T
    wpad[:, P:P2] = W[:, D:].T
    wpk = np.ascontiguousarray(
        wpad.reshape(2, 128, 128).transpose(1, 0, 2).reshape(128, 256)
    ).astype(bf16)

    hs_bf = hs.astype(bf16)
    in_maps = []
    for c in range(NCORES):
        sl = slice(c * BLOC, (c + 1) * BLOC)
        # compacted table: hs_m[l, i, j, d] = hs[l, b, uq_pad[l, b, j], d]
        hs_m = np.take_along_axis(hs_bf[:, sl], uq_pad[:, sl, :, None], axis=2)
        # [NB, KR, 2, 128] -> [NG, 128, 2, G, KR]
        hst = np.ascontiguousarray(
            hs_m.reshape(NG, G, KR, 2, 128).transpose(0, 4, 3, 1, 2)
            .reshape(NG, 128, 2 * G * KR))
        in_maps.append({"hst": hst, "wpk": wpk})
    return KR, in_maps, j_sub, j_obj


def kernel(hs, src_indices, tgt_perm, relationships, W_pred, b_pred):
    if "concourse" not in sys.modules:
        try:
            import concourse  # noqa: F401
        except ImportError:
            sys.path.insert(0, "/opt/trn_rl_repo")
    from concourse import bass_utils

    KR, in_maps, j_sub, j_obj = _host_prepare(
        hs, src_indices, tgt_perm, relationships, W_pred, b_pred)
    if _CACHE.get("KR") != KR:
        _CACHE["nc"] = _build_program(KR)
        _CACHE["KR"] = KR
    nc = _CACHE["nc"]

    res = bass_utils.run_bass_kernel_spmd(nc, in_maps, list(range(NCORES)))

    b = np.asarray(b_pred, dtype=np.float32)
    outs = []
    for c in range(NCORES):
        ab = res.results[c]["outab"]                  # [NG, 128, G*KR] bf16
        ab = ab.astype(np.float32).reshape(NG, 128, G, KR)
        ab = ab.transpose(0, 2, 1, 3).reshape(L, BLOC, 128, KR)
        sl = slice(c * BLOC, (c + 1) * BLOC)
        ks = j_sub[:, sl, None, :]                    # [L, BLOC, 1, R]
        ko = j_obj[:, sl, None, :]
        a_half = np.take_along_axis(ab[:, :, :P, :],
                                    np.broadcast_to(ks, (L, BLOC, P, R)),
                                    axis=3)           # [L, BLOC, P, R]
        b_half = np.take_along_axis(ab[:, :, P:P2, :],
                                    np.broadcast_to(ko, (L, BLOC, P, R)),
                                    axis=3)
        logits = (a_half + b_half).transpose(0, 1, 3, 2) + b
        outs.append(np.ascontiguousarray(logits))
    return np.concatenate(outs, axis=1)
